# revision 9
# baseline (speedup 1.0000x reference)
"""Trainium2 Bass kernel for the 4-layer Llama-style reference model.

Sharding (8 NeuronCores):
  - 2 batch groups of 4 cores (core c: batch b=c//4, quarter p=c%4).
  - Trunk: tensor-parallel over heads (2 heads per core) for attention;
    sequence-sharded (4-way within the batch group) residual stream, norms
    and FFN.  Per layer: AllGather of the normalized residual (bf16) before
    QKV, ReduceScatter (fp32) of the attention output partials.
  - Final head: vocab-sharded output projection (8000 cols per core).
  - Embedding: vocab-sharded table (4000 rows/core), on-device indirect-DMA
    gather with OOB-skip, 8-way ReduceScatter distributes x0 slices.

Layouts: activations feeding matmuls are kept feature-major ([d, s], d on
partitions); RoPE runs on DVE/GPSIMD with even/odd feature pairs separated
into distinct partition tiles via host-side permutation of the projection
weight columns (and matching permutation of Wo rows / v columns).

Numerics: bf16 matmul inputs with fp32 PSUM accumulation; fp32 residual
stream, softmax statistics and logits.
"""

import math

import numpy as np
import ml_dtypes

from concourse import bass, bacc, mybir, tile
from concourse import bass_utils
from concourse.masks import make_identity

BF16 = mybir.dt.bfloat16
F32 = mybir.dt.float32
I32 = mybir.dt.int32

NCORES = 8
GROUPS = [[0, 1, 2, 3], [4, 5, 6, 7]]
ALLG = [list(range(NCORES))]

D = 256            # d_model
S = 1024           # context window
H = 8              # heads
V = 32000          # vocab
L = 4              # layers
B = 2              # batch
SL = 256           # sequence slice per core (S / 4)
VSH = V // NCORES  # embed-table rows per core (4000)
VSL = V // 4       # vocab slice of the output projection per core (8000)
NEG_SCALE = 1.0 / 16.0  # 1/sqrt(D)

_bf = ml_dtypes.bfloat16

_CACHED = {}


# ----------------------------------------------------------------------------
# IR emission
# ----------------------------------------------------------------------------

def _valid_ic(jb):
    """i-chunks (512 wide) intersecting the causal region for j-tile jb."""
    return [0, 1] if jb < 4 else [1]


def _e_col(jb, ic):
    """Column offset of block (jb, ic) in the packed E tile."""
    if jb < 4:
        return jb * 1024 + ic * 512
    return 4096 + (jb - 4) * 512  # only ic == 1 stored


E_WIDTH = 6144  # 4*1024 + 4*512


def build_nc():
    nc = bacc.Bacc("TRN2", target_bir_lowering=False, debug=False,
                   num_devices=NCORES)

    # ---- I/O declarations (per-core) ----
    d_tok = nc.dram_tensor("tok", [128, 16], I32, kind="ExternalInput")
    d_emb = nc.dram_tensor("emb", [VSH, D], F32, kind="ExternalInput")
    d_w = {}
    for l in range(L):
        for j in range(2):
            for nm in ("wq", "wk", "wv"):
                d_w[f"{nm}{l}h{j}"] = nc.dram_tensor(
                    f"{nm}{l}h{j}", [D, D], BF16, kind="ExternalInput")
        d_w[f"wo{l}"] = nc.dram_tensor(f"wo{l}", [2 * D, D], BF16,
                                       kind="ExternalInput")
        for nm in ("ffw", "wg", "wl"):
            d_w[f"{nm}{l}"] = nc.dram_tensor(f"{nm}{l}", [D, D], BF16,
                                             kind="ExternalInput")
    for nm in ("fw", "fwg", "fwl"):
        d_w[nm] = nc.dram_tensor(nm, [D, D], BF16, kind="ExternalInput")
    d_outw = nc.dram_tensor("outw", [D, VSL], BF16, kind="ExternalInput")
    d_cosT = nc.dram_tensor("cosT", [128, S], BF16, kind="ExternalInput")
    d_sinT = nc.dram_tensor("sinT", [128, S], BF16, kind="ExternalInput")
    d_cosS = nc.dram_tensor("cosS", [128, 8, 128], BF16, kind="ExternalInput")
    d_sinS = nc.dram_tensor("sinS", [128, 8, 128], BF16, kind="ExternalInput")
    d_mask = nc.dram_tensor("maskT", [128, 4, 512], BF16, kind="ExternalInput")
    d_out = nc.dram_tensor("out", [S, VSL], F32, kind="ExternalOutput")

    with tile.TileContext(nc) as tc:
        with (
            tc.tile_pool(name="wp", bufs=1) as wp,
            tc.tile_pool(name="ap", bufs=1) as ap,
            tc.tile_pool(name="ap2", bufs=2) as ap2,
            tc.tile_pool(name="rt", bufs=3) as rt,
            tc.tile_pool(name="ps", bufs=8, space="PSUM") as ps,
            tc.tile_pool(name="dp", bufs=2, space="DRAM") as dp,
        ):
            _emit(nc, tc, wp, ap, ap2, rt, ps, dp, d_tok, d_emb, d_w, d_outw,
                  d_cosT, d_sinT, d_cosS, d_sinS, d_mask, d_out)

    nc.compile()
    return nc


def _emit(nc, tc, wp, ap, ap2, rt, ps, dp, d_tok, d_emb, d_w, d_outw,
          d_cosT, d_sinT, d_cosS, d_sinS, d_mask, d_out):
    TT = nc.vector.tensor_tensor
    ACT = nc.scalar.activation
    MM = nc.tensor.matmul
    AF = mybir.ActivationFunctionType
    OP = mybir.AluOpType

    # ---- persistent constants / weights in SBUF ----
    def load(name, shape, dtype, src):
        t = wp.tile(shape, dtype, tag=name)
        nc.sync.dma_start(out=t[:], in_=src)
        return t

    tok = load("tok", [128, 16], I32, d_tok[:, :])
    cosT = load("cosT", [128, S], BF16, d_cosT[:, :])
    sinT = load("sinT", [128, S], BF16, d_sinT[:, :])
    cosS = load("cosS", [128, 8, 128], BF16, d_cosS[:, :, :])
    sinS = load("sinS", [128, 8, 128], BF16, d_sinS[:, :, :])
    maskT = load("maskT", [128, 4, 512], BF16, d_mask[:, :, :])

    ident = wp.tile([128, 128], F32, tag="ident")
    make_identity(nc, ident[:])
    # constants used as activation bias APs
    for cval in (0.0, 1e-6):
        ct = wp.tile([128, 1], F32, tag=f"const{cval}")
        nc.vector.memset(ct[:], cval)
        nc.const_aps.aps[(F32, cval)] = ct[:]
    ones_cf = wp.tile([128, 1], F32, tag="ones_cf")
    nc.vector.memset(ones_cf[:], 1.0)
    ones_cb = wp.tile([128, 1], BF16, tag="ones_cb")
    nc.vector.memset(ones_cb[:], 1.0)
    ones_r = wp.tile([1, 128], F32, tag="ones_r")
    nc.vector.memset(ones_r[:], 1.0)

    def load_dd(name):
        """[D, D] weight -> SBUF [128, 2*D], chunk dch at [:, dch*D:(dch+1)*D]."""
        t = wp.tile([128, 2 * D], BF16, tag=name)
        for dch in range(2):
            nc.sync.dma_start(out=t[:, dch * D:(dch + 1) * D],
                              in_=d_w[name][dch * 128:(dch + 1) * 128, :])
        return t

    w = {}
    for l in range(L):
        for j in range(2):
            for nm in ("wq", "wk", "wv"):
                w[f"{nm}{l}h{j}"] = load_dd(f"{nm}{l}h{j}")
        t = wp.tile([128, 4 * D], BF16, tag=f"wo{l}")
        for dch in range(4):
            nc.sync.dma_start(out=t[:, dch * D:(dch + 1) * D],
                              in_=d_w[f"wo{l}"][dch * 128:(dch + 1) * 128, :])
        w[f"wo{l}"] = t
        for nm in ("ffw", "wg", "wl"):
            w[f"{nm}{l}"] = load_dd(f"{nm}{l}")
    for nm in ("fw", "fwg", "fwl"):
        w[nm] = load_dd(nm)

    # ---- embedding: gather from local table shard, 8-way ReduceScatter ----
    ers_in = dp.tile([2048, D], F32, tag="ers_in")
    for t16 in range(16):
        g = ap2.tile([128, D], F32, tag="x0g")
        nc.gpsimd.memset(g[:], 0.0)
        nc.gpsimd.indirect_dma_start(
            out=g[:], out_offset=None, in_=d_emb[:, :],
            in_offset=bass.IndirectOffsetOnAxis(ap=tok[:, t16:t16 + 1], axis=0),
            bounds_check=VSH - 1, oob_is_err=False)
        nc.sync.dma_start(out=ers_in[t16 * 128:(t16 + 1) * 128, :], in_=g[:])
    ers_out = dp.tile([SL, D], F32, tag="ers_out")
    nc.gpsimd.collective_compute(
        "ReduceScatter", OP.add, replica_groups=ALLG,
        ins=[ers_in[:].opt()], outs=[ers_out[:].opt()])

    xseq = ap.tile([128, 2 * D], F32, tag="xseq")  # [s-tile si, d]
    for si in range(2):
        nc.sync.dma_start(out=xseq[:, si * D:(si + 1) * D],
                          in_=ers_out[si * 128:(si + 1) * 128, :])
    # transpose to feature-major xT [128, dch*SL + s']
    xT = ap2.tile([128, 2 * SL], F32, tag="xT")
    for si in range(2):
        for dj in range(2):
            pt = ps.tile([128, 128], F32, tag="ps")
            nc.tensor.transpose(pt[:], xseq[:, si * D + dj * 128:
                                            si * D + dj * 128 + 128], ident[:])
            nc.scalar.activation(xT[:, dj * SL + si * 128:dj * SL + si * 128 + 128],
                                 pt[:], AF.Copy)

    # ---- helpers ----
    def rmsnorm(x_in, tag):
        """feature-major rmsnorm on the local slice. returns (xn_f32, xn_bf16)."""
        sq = ap.tile([128, 2 * SL], F32, tag="sq")
        TT(sq[:], x_in[:], x_in[:], op=OP.mult)
        msp = ps.tile([1, SL], F32, tag="ps")
        for dch in range(2):
            MM(msp[:], lhsT=ones_cf[:], rhs=sq[:, dch * SL:(dch + 1) * SL],
               start=(dch == 0), stop=(dch == 1))
        lnm = ap.tile([1, SL], F32, tag="lnm")
        ACT(lnm[:], msp[:], AF.Ln, bias=1e-6, scale=1.0 / D)
        rstd = ap.tile([1, SL], F32, tag="rstd")
        ACT(rstd[:], lnm[:], AF.Exp, scale=-0.5)
        bc = ps.tile([128, SL], F32, tag="ps")
        MM(bc[:], lhsT=ones_r[:], rhs=rstd[:], start=True, stop=True)
        xn = ap.tile([128, 2 * SL], F32, tag=tag)
        xnb = ap.tile([128, 2 * SL], BF16, tag=tag + "b")
        for dch in range(2):
            TT(xn[:, dch * SL:(dch + 1) * SL], x_in[:, dch * SL:(dch + 1) * SL],
               bc[:], op=OP.mult)
            nc.vector.tensor_copy(xnb[:, dch * SL:(dch + 1) * SL],
                                  xn[:, dch * SL:(dch + 1) * SL])
        return xn, xnb

    def allgather_feat(src_bf16, tag):
        """AllGather the [128, 2*SL] bf16 feature-major slice within the batch
        group -> [128, 2*S] feature-major full-batch tile."""
        agi = dp.tile([2 * 128, SL], BF16, tag=tag + "_agi")
        for dch in range(2):
            nc.sync.dma_start(out=agi[dch * 128:(dch + 1) * 128, :],
                              in_=src_bf16[:, dch * SL:(dch + 1) * SL])
        ago = dp.tile([4, 2 * 128, SL], BF16, tag=tag + "_ago")
        nc.gpsimd.collective_compute(
            "AllGather", OP.bypass, replica_groups=GROUPS,
            ins=[agi[:].opt()], outs=[ago[:].opt()])
        full = ap2.tile([128, 2 * S], BF16, tag=tag)
        for dch in range(2):
            for g4 in range(4):
                nc.sync.dma_start(
                    out=full[:, dch * S + g4 * SL:dch * S + (g4 + 1) * SL],
                    in_=ago[g4, dch * 128:(dch + 1) * 128, :])
        return full

    def rope_feat(raw, out, tag):
        """RoPE on a feature-major [128, 2*S] bf16 tensor (A block then B)."""
        A, Bq = raw[:, 0:S], raw[:, S:2 * S]
        t1 = rt.tile([128, S], BF16, tag="rtmp")
        t2 = rt.tile([128, S], BF16, tag="rtmp")
        TT(t1[:], A, cosT[:], op=OP.mult)
        TT(t2[:], Bq, sinT[:], op=OP.mult)
        TT(out[:, 0:S], t1[:], t2[:], op=OP.add)
        t3 = rt.tile([128, S], BF16, tag="rtmp")
        t4 = rt.tile([128, S], BF16, tag="rtmp")
        TT(t3[:], Bq, cosT[:], op=OP.mult)
        TT(t4[:], A, sinT[:], op=OP.mult)
        TT(out[:, S:2 * S], t3[:], t4[:], op=OP.subtract)

    def ffn(x_bf, wf, wg_, wl_, out_dtype, out_tag):
        """x_bf [128, 2*SL] bf16 -> swiglu(x@wf) [128, 2*SL] feature-major."""
        hps = []
        for ech in range(2):
            hp = ps.tile([128, SL], F32, tag="ps")
            for dch in range(2):
                MM(hp[:], lhsT=wf[:, dch * D + ech * 128:dch * D + ech * 128 + 128],
                   rhs=x_bf[:, dch * SL:(dch + 1) * SL],
                   start=(dch == 0), stop=(dch == 1))
            hps.append(hp)
        hb = ap.tile([128, 2 * SL], BF16, tag="hb")
        for ech in range(2):
            ACT(hb[:, ech * SL:(ech + 1) * SL], hps[ech][:], AF.Copy)
        gps, lps = [], []
        for wmat, acc in ((wg_, gps), (wl_, lps)):
            for ech in range(2):
                p = ps.tile([128, SL], F32, tag="ps")
                for dch in range(2):
                    MM(p[:], lhsT=wmat[:, dch * D + ech * 128:dch * D + ech * 128 + 128],
                       rhs=hb[:, dch * SL:(dch + 1) * SL],
                       start=(dch == 0), stop=(dch == 1))
                acc.append(p)
        fo = ap.tile([128, 2 * SL], out_dtype, tag=out_tag)
        eN = ap.tile([128, 2 * SL], F32, tag="eN")
        den = ap.tile([128, 2 * SL], F32, tag="den")
        rc2 = ap.tile([128, 2 * SL], F32, tag="rc2")
        swt = ap.tile([128, 2 * SL], F32, tag="swt")
        for ech in range(2):
            sl = slice(ech * SL, (ech + 1) * SL)
            ACT(eN[:, sl], gps[ech][:], AF.Exp, scale=-1.0)
            nc.vector.tensor_scalar_add(den[:, sl], eN[:, sl], 1.0)
            nc.vector.reciprocal(rc2[:, sl], den[:, sl])
            nc.vector.scalar_tensor_tensor(
                out=swt[:, sl], in0=gps[ech][:], scalar=1.0, in1=rc2[:, sl],
                op0=OP.mult, op1=OP.mult)
            TT(fo[:, sl], swt[:, sl], lps[ech][:], op=OP.mult)
        return fo

    # ---- transformer layers ----
    x_cur = xT
    for l in range(L):
        xn, xnb = rmsnorm(x_cur, f"xn")
        xnF = allgather_feat(xnb, "xnF")

        attnS = ap.tile([128, 2 * S], F32, tag="attnS")
        for j in range(2):  # heads
            # --- q, k projections (feature-major, weight-stationary) ---
            qraw = ap.tile([128, 2 * S], BF16, tag="qraw")
            kraw = ap.tile([128, 2 * S], BF16, tag="kraw")
            for wmat, dst in ((w[f"wq{l}h{j}"], qraw), (w[f"wk{l}h{j}"], kraw)):
                for ech in range(2):
                    for sch in range(2):
                        p = ps.tile([128, 512], F32, tag="ps")
                        for dch in range(2):
                            MM(p[:],
                               lhsT=wmat[:, dch * D + ech * 128:dch * D + ech * 128 + 128],
                               rhs=xnF[:, dch * S + sch * 512:dch * S + (sch + 1) * 512],
                               start=(dch == 0), stop=(dch == 1))
                        ACT(dst[:, ech * S + sch * 512:ech * S + (sch + 1) * 512],
                            p[:], AF.Copy)
            qT = ap.tile([128, 2 * S], BF16, tag="qT")
            kT = ap.tile([128, 2 * S], BF16, tag="kT")
            rope_feat(qraw, qT, "q")
            rope_feat(kraw, kT, "k")

            # --- v projection (sequence-major, xn-stationary) ---
            vraw = ap.tile([128, 8, D], BF16, tag="vraw")
            for st in range(8):
                p = ps.tile([128, D], F32, tag="ps")
                for dch in range(2):
                    MM(p[:],
                       lhsT=xnF[:, dch * S + st * 128:dch * S + (st + 1) * 128],
                       rhs=w[f"wv{l}h{j}"][:, dch * D:(dch + 1) * D],
                       start=(dch == 0), stop=(dch == 1))
                nc.vector.tensor_copy(vraw[:, st, :], p[:])
            # RoPE on v (gpsimd; pairs are columns 0:128 / 128:256)
            vR = ap.tile([128, 8, D], BF16, tag="vR")
            vA, vB = vraw[:, :, 0:128], vraw[:, :, 128:256]
            g1 = rt.tile([128, 8, 128], BF16, tag="vtmp")
            g2 = rt.tile([128, 8, 128], BF16, tag="vtmp")
            nc.gpsimd.tensor_tensor(g1[:], vA, cosS[:], op=OP.mult)
            nc.gpsimd.tensor_tensor(g2[:], vB, sinS[:], op=OP.mult)
            nc.gpsimd.tensor_tensor(vR[:, :, 0:128], g1[:], g2[:], op=OP.add)
            g3 = rt.tile([128, 8, 128], BF16, tag="vtmp")
            g4 = rt.tile([128, 8, 128], BF16, tag="vtmp")
            nc.gpsimd.tensor_tensor(g3[:], vB, cosS[:], op=OP.mult)
            nc.gpsimd.tensor_tensor(g4[:], vA, sinS[:], op=OP.mult)
            nc.gpsimd.tensor_tensor(vR[:, :, 128:256], g3[:], g4[:],
                                    op=OP.subtract)

            # --- scores^T blocks + exp + causal mask ---
            E = ap.tile([128, E_WIDTH], BF16, tag="E")
            for jb in range(8):
                scp = {ic: ps.tile([128, 512], F32, tag="ps",
                                   name=f"scp{l}_{j}_{jb}_{ic}")
                       for ic in _valid_ic(jb)}
                for dch in range(2):
                    for ic in _valid_ic(jb):
                        MM(scp[ic][:],
                           lhsT=qT[:, dch * S + jb * 128:dch * S + (jb + 1) * 128],
                           rhs=kT[:, dch * S + ic * 512:dch * S + (ic + 1) * 512],
                           start=(dch == 0), stop=(dch == 1))
                for ic in _valid_ic(jb):
                    col = _e_col(jb, ic)
                    ACT(E[:, col:col + 512], scp[ic][:], AF.Exp, scale=NEG_SCALE)
                # diagonal block mask (block ic == jb//4)
                cold = _e_col(jb, jb // 4)
                TT(E[:, cold:cold + 512], E[:, cold:cold + 512],
                   maskT[:, jb % 4, :], op=OP.mult)

            # --- row sums over j (ones-matmul), broadcast, reciprocal ---
            rsS = ap.tile([1, S], F32, tag="rsS")
            rcp = ap.tile([128, S], F32, tag="rcp")
            for ic in range(2):
                rp = ps.tile([1, 512], F32, tag="ps")
                jbs = [jb for jb in range(8) if ic in _valid_ic(jb)]
                for n, jb in enumerate(jbs):
                    col = _e_col(jb, ic)
                    MM(rp[:], lhsT=ones_cb[:], rhs=E[:, col:col + 512],
                       start=(n == 0), stop=(n == len(jbs) - 1))
                ACT(rsS[:, ic * 512:(ic + 1) * 512], rp[:], AF.Copy)
                bp = ps.tile([128, 512], F32, tag="ps")
                MM(bp[:], lhsT=ones_r[:], rhs=rsS[:, ic * 512:(ic + 1) * 512],
                   start=True, stop=True)
                nc.vector.reciprocal(rcp[:, ic * 512:(ic + 1) * 512], bp[:])

            # --- attention * V (o feature-major), normalize ---
            avp = {(dpr, ic): ps.tile([128, 512], F32, tag="ps",
                                      name=f"avp{l}_{j}_{dpr}_{ic}")
                   for dpr in range(2) for ic in range(2)}
            for jb in range(8):
                for dpr in range(2):
                    for ic in _valid_ic(jb):
                        col = _e_col(jb, ic)
                        MM(avp[(dpr, ic)][:],
                           lhsT=vR[:, jb, dpr * 128:(dpr + 1) * 128],
                           rhs=E[:, col:col + 512],
                           start=(jb == 0), stop=(jb == (3 if ic == 0 else 7)))
            oN = ap.tile([128, 2 * S], BF16, tag="oN")
            for dpr in range(2):
                for ic in range(2):
                    sl = slice(dpr * S + ic * 512, dpr * S + (ic + 1) * 512)
                    TT(oN[:, sl], avp[(dpr, ic)][:],
                       rcp[:, ic * 512:(ic + 1) * 512], op=OP.mult)

            # --- Wo partial (accumulate heads in SBUF) ---
            for ech in range(2):
                for sch in range(2):
                    p = ps.tile([128, 512], F32, tag="ps")
                    for dpr in range(2):
                        MM(p[:],
                           lhsT=w[f"wo{l}"][:, (2 * j + dpr) * D + ech * 128:
                                            (2 * j + dpr) * D + ech * 128 + 128],
                           rhs=oN[:, dpr * S + sch * 512:dpr * S + (sch + 1) * 512],
                           start=(dpr == 0), stop=(dpr == 1))
                    sl = slice(ech * S + sch * 512, ech * S + (sch + 1) * 512)
                    if j == 0:
                        ACT(attnS[:, sl], p[:], AF.Copy)
                    else:
                        TT(attnS[:, sl], attnS[:, sl], p[:], op=OP.add)

        # --- ReduceScatter attention output, residual, rms2, FFN ---
        rsi = dp.tile([4, 2 * 128, SL], F32, tag="arsi")
        for g4 in range(4):
            for ech in range(2):
                nc.sync.dma_start(
                    out=rsi[g4, ech * 128:(ech + 1) * 128, :],
                    in_=attnS[:, ech * S + g4 * SL:ech * S + (g4 + 1) * SL])
        rso = dp.tile([2 * 128, SL], F32, tag="arso")
        nc.gpsimd.collective_compute(
            "ReduceScatter", OP.add, replica_groups=GROUPS,
            ins=[rsi[:].opt()], outs=[rso[:].opt()])
        aS = ap.tile([128, 2 * SL], F32, tag="aS")
        for ech in range(2):
            nc.sync.dma_start(out=aS[:, ech * SL:(ech + 1) * SL],
                              in_=rso[ech * 128:(ech + 1) * 128, :])
        x2 = ap2.tile([128, 2 * SL], F32, tag="xT")
        TT(x2[:], xn[:], aS[:], op=OP.add)

        xn2, xn2b = rmsnorm(x2, "xn2")
        fo = ffn(xn2b, w[f"ffw{l}"], w[f"wg{l}"], w[f"wl{l}"], F32, "fo")
        x3 = ap2.tile([128, 2 * SL], F32, tag="xT")
        TT(x3[:], xn2[:], fo[:], op=OP.add)
        x_cur = x3

    # ---- final head ----
    xfb = ap.tile([128, 2 * SL], BF16, tag="xfb")
    nc.vector.tensor_copy(xfb[:], x_cur[:])
    fT = ffn(xfb, w["fw"], w["fwg"], w["fwl"], BF16, "fT")
    fF = allgather_feat(fT, "fF")

    VHW = VSL // 2   # 4000 vocab cols per half
    VC = 500         # free-dim chunk (8 per half)
    owc = ap.tile([128, 2 * VHW], BF16, tag="owc")
    for vh in range(2):
        for dch in range(2):
            nc.sync.dma_start(
                out=owc[:, dch * VHW:(dch + 1) * VHW],
                in_=d_outw[dch * 128:(dch + 1) * 128,
                           vh * VHW:(vh + 1) * VHW])
        for st in range(8):
            pls = [ps.tile([128, VC], F32, tag="ps", name=f"pls{vh}_{st}_{i}")
                   for i in range(8)]
            for dch in range(2):
                for vc in range(8):
                    MM(pls[vc][:],
                       lhsT=fF[:, dch * S + st * 128:dch * S + (st + 1) * 128],
                       rhs=owc[:, dch * VHW + vc * VC:dch * VHW + (vc + 1) * VC],
                       start=(dch == 0), stop=(dch == 1))
            for vc in range(8):
                oc = ap2.tile([128, VC], F32, tag="oc")
                if vc % 2 == 0:
                    ACT(oc[:], pls[vc][:], AF.Copy)
                else:
                    nc.vector.tensor_copy(oc[:], pls[vc][:])
                nc.sync.dma_start(
                    out=d_out[st * 128:(st + 1) * 128,
                              vh * VHW + vc * VC:vh * VHW + (vc + 1) * VC],
                    in_=oc[:])


# ----------------------------------------------------------------------------
# Host side
# ----------------------------------------------------------------------------

def _rope_tables_np():
    i = np.arange(D // 2, dtype=np.float32)
    theta = (10000.0 ** (-2.0 * (i - 1.0) / D)).astype(np.float32)
    ang = np.arange(S, dtype=np.float32)[:, None] * theta[None, :]
    return np.cos(ang).astype(np.float32), np.sin(ang).astype(np.float32)


def _prep_inputs(tokens, params):
    """Build the 8 per-core input maps."""
    perm = np.concatenate([np.arange(0, D, 2), np.arange(1, D, 2)])
    cos, sin = _rope_tables_np()  # [S, 128]

    def b16(x):
        return np.asarray(x, dtype=np.float32).astype(_bf)

    cosT = b16(cos.T)                      # [128, S]
    sinT = b16(sin.T)
    cosSe = b16(cos.reshape(8, 128, 128).transpose(1, 0, 2))  # [p, st, i]
    sinSe = b16(sin.reshape(8, 128, 128).transpose(1, 0, 2))
    # mask[p, o, f] = 1.0 if f >= p + o*128 else 0
    f = np.arange(512)[None, None, :]
    p = np.arange(128)[:, None, None]
    o = np.arange(4)[None, :, None]
    maskT = (f >= p + o * 128).astype(np.float32).astype(_bf)

    def z(x):
        assert np.all(np.asarray(x) == 0.0), "nonzero bias unsupported"

    def one(x):
        assert np.all(np.asarray(x) == 1.0), "non-unit rms scale unsupported"

    lp = params["layers"]
    for pl in lp:
        for k in ("bq", "bk", "bv", "bo", "ff_b", "sg_bg", "sg_bl"):
            z(pl[k])
        one(pl["rms1"])
        one(pl["rms2"])
    for k in ("f_b", "f_sg_bg", "f_sg_bl", "out_b"):
        z(params[k])

    emb = np.asarray(params["embed"], dtype=np.float32)
    tokens = np.asarray(tokens, dtype=np.int32).reshape(-1)  # [2048]

    in_maps = []
    for c in range(NCORES):
        p4 = c % 4
        m = {}
        loc = tokens - c * VSH
        loc = np.where((loc >= 0) & (loc < VSH), loc, VSH).astype(np.int32)
        m["tok"] = loc.reshape(16, 128).T.copy()  # [p, t16]
        m["emb"] = emb[c * VSH:(c + 1) * VSH]
        for l in range(L):
            pl = lp[l]
            for j in range(2):
                h = 2 * p4 + j
                m[f"wq{l}h{j}"] = b16(np.asarray(pl["Wq"])[h][:, perm])
                m[f"wk{l}h{j}"] = b16(np.asarray(pl["Wk"])[h][:, perm])
                m[f"wv{l}h{j}"] = b16(np.asarray(pl["Wv"])[h][:, perm])
            wo = np.asarray(pl["Wo"])  # [H*D, D]
            blocks = []
            for j in range(2):
                h = 2 * p4 + j
                blocks.append(wo[h * D:(h + 1) * D][perm, :])
            m[f"wo{l}"] = b16(np.concatenate(blocks, axis=0))
            m[f"ffw{l}"] = b16(pl["ff_W"])
            m[f"wg{l}"] = b16(pl["sg_Wg"])
            m[f"wl{l}"] = b16(pl["sg_Wl"])
        m["fw"] = b16(params["f_W"])
        m["fwg"] = b16(params["f_sg_Wg"])
        m["fwl"] = b16(params["f_sg_Wl"])
        m["outw"] = b16(np.asarray(params["out_W"])[:, p4 * VSL:(p4 + 1) * VSL])
        m["cosT"] = cosT
        m["sinT"] = sinT
        m["cosS"] = cosSe
        m["sinS"] = sinSe
        m["maskT"] = maskT
        in_maps.append(m)
    return in_maps


def _get_runner():
    """Build (once) a cached jitted SPMD executable for the compiled module.

    Mirrors bass2jax.run_bass_via_pjrt's multi-core path, but keeps the jitted
    function and the (device-resident, undonated) zero output buffers alive so
    repeat calls skip retracing and the 262MB zero upload.  Safe because the
    kernel writes every element of its output tensor.
    """
    if "runner" in _CACHED:
        return _CACHED["runner"]
    import jax
    from jax.sharding import Mesh, PartitionSpec
    from jax.experimental.shard_map import shard_map
    from concourse import bass2jax, mybir as _mb

    if "nc" not in _CACHED:
        _CACHED["nc"] = build_nc()
    nc = _CACHED["nc"]
    bass2jax.install_neuronx_cc_hook()
    assert nc.dbg_addr is None
    part_name = (nc.partition_id_tensor.name
                 if nc.partition_id_tensor is not None else None)

    in_names, out_names, out_avals, zero_outs = [], [], [], []
    for alloc in nc.m.functions[0].allocations:
        if not isinstance(alloc, _mb.MemoryLocationSet):
            continue
        name = alloc.memorylocations[0].name
        if alloc.kind == "ExternalInput":
            if name != part_name:
                in_names.append(name)
        elif alloc.kind == "ExternalOutput":
            out_names.append(name)
            shape = tuple(alloc.tensor_shape)
            dtype = _mb.dt.np(alloc.dtype)
            out_avals.append(jax.core.ShapedArray(shape, dtype))
            zero_outs.append(np.zeros((NCORES * shape[0], *shape[1:]), dtype))
    n_params = len(in_names)
    all_names = in_names + out_names
    if part_name is not None:
        all_names = all_names + [part_name]

    def _body(*args):
        operands = list(args)
        if part_name is not None:
            operands.append(bass2jax.partition_id_tensor())
        outs = bass2jax._bass_exec_p.bind(
            *operands, out_avals=tuple(out_avals), in_names=tuple(all_names),
            out_names=tuple(out_names), lowering_input_output_aliases=(),
            sim_require_finite=True, sim_require_nnan=True, nc=nc)
        return tuple(outs)

    devices = jax.devices()[:NCORES]
    mesh = Mesh(np.asarray(devices), ("core",))
    nargs = n_params + len(out_names)
    sharded = jax.jit(
        shard_map(_body, mesh=mesh, in_specs=(PartitionSpec("core"),) * nargs,
                  out_specs=(PartitionSpec("core"),) * len(out_names),
                  check_rep=False),
        keep_unused=True)
    sharding = jax.sharding.NamedSharding(mesh, PartitionSpec("core"))
    dev_zeros = [jax.device_put(z, sharding) for z in zero_outs]

    def run(in_maps, dev_inputs=None):
        if dev_inputs is None:
            dev_inputs = put_inputs(in_maps)
        out_arrs = sharded(*dev_inputs, *dev_zeros)
        return out_arrs

    def put_inputs(in_maps):
        concat = [np.concatenate([np.asarray(in_maps[c][nm])
                                  for c in range(NCORES)], axis=0)
                  for nm in in_names]
        return [jax.device_put(a, sharding) for a in concat]

    _CACHED["runner"] = (run, put_inputs, out_names, out_avals)
    return _CACHED["runner"]


def kernel(tokens, params):
    run, put_inputs, out_names, out_avals = _get_runner()
    in_maps = _prep_inputs(tokens, params)
    out_arrs = run(in_maps)
    oi = out_names.index("out")
    full = np.asarray(out_arrs[oi]).reshape(NCORES, S, VSL)
    out = np.empty((B, S, V), dtype=np.float32)
    for c in range(NCORES):
        b, p4 = c // 4, c % 4
        out[b, :, p4 * VSL:(p4 + 1) * VSL] = full[c]
    return out


# revision 12
# speedup vs baseline: 1.2223x; 1.2223x over previous
"""Trainium2 Bass kernel for the 4-layer Llama-style reference model.

Sharding (8 NeuronCores):
  - 2 batch groups of 4 cores (core c: batch b=c//4, quarter p=c%4).
  - Trunk: tensor-parallel over heads (2 heads per core) for attention;
    sequence-sharded (4-way within the batch group) residual stream, norms
    and FFN.  Per layer: AllGather of the normalized residual (bf16) before
    QKV, ReduceScatter (fp32) of the attention output partials.
  - Final head: vocab-sharded output projection (8000 cols per core).
  - Embedding: vocab-sharded table (4000 rows/core), on-device indirect-DMA
    gather with OOB-skip, 8-way ReduceScatter distributes x0 slices.

Layouts: activations feeding matmuls are kept feature-major ([d, s], d on
partitions); RoPE runs on DVE/GPSIMD with even/odd feature pairs separated
into distinct partition tiles via host-side permutation of the projection
weight columns (and matching permutation of Wo rows / v columns).

Numerics: bf16 matmul inputs with fp32 PSUM accumulation; fp32 residual
stream, softmax statistics and logits.
"""

import math

import numpy as np
import ml_dtypes

from concourse import bass, bacc, mybir, tile
from concourse import bass_utils
from concourse.masks import make_identity

BF16 = mybir.dt.bfloat16
F32 = mybir.dt.float32
I32 = mybir.dt.int32

NCORES = 8
GROUPS = [[0, 1, 2, 3], [4, 5, 6, 7]]
ALLG = [list(range(NCORES))]

D = 256            # d_model
S = 1024           # context window
H = 8              # heads
V = 32000          # vocab
L = 4              # layers
B = 2              # batch
SL = 256           # sequence slice per core (S / 4)
VSH = V // NCORES  # embed-table rows per core (4000)
VSL = V // 4       # vocab slice of the output projection per core (8000)
NEG_SCALE = 1.0 / 16.0  # 1/sqrt(D)

_bf = ml_dtypes.bfloat16

_CACHED = {}


# ----------------------------------------------------------------------------
# IR emission
# ----------------------------------------------------------------------------

def _valid_ic(jb):
    """i-chunks (512 wide) intersecting the causal region for j-tile jb."""
    return [0, 1] if jb < 4 else [1]


def _e_col(jb, ic):
    """Column offset of block (jb, ic) in the packed E tile."""
    if jb < 4:
        return jb * 1024 + ic * 512
    return 4096 + (jb - 4) * 512  # only ic == 1 stored


E_WIDTH = 6144  # 4*1024 + 4*512


def build_nc():
    nc = bacc.Bacc("TRN2", target_bir_lowering=False, debug=False,
                   num_devices=NCORES)

    # ---- I/O declarations (per-core) ----
    d_tok = nc.dram_tensor("tok", [128, 16], I32, kind="ExternalInput")
    d_emb = nc.dram_tensor("emb", [VSH, D], F32, kind="ExternalInput")
    d_w = {}
    for l in range(L):
        for j in range(2):
            for nm in ("wq", "wk", "wv"):
                d_w[f"{nm}{l}h{j}"] = nc.dram_tensor(
                    f"{nm}{l}h{j}", [D, D], BF16, kind="ExternalInput")
        d_w[f"wo{l}"] = nc.dram_tensor(f"wo{l}", [2 * D, D], BF16,
                                       kind="ExternalInput")
        for nm in ("ffw", "wg", "wl"):
            d_w[f"{nm}{l}"] = nc.dram_tensor(f"{nm}{l}", [D, D], BF16,
                                             kind="ExternalInput")
    for nm in ("fw", "fwg", "fwl"):
        d_w[nm] = nc.dram_tensor(nm, [D, D], BF16, kind="ExternalInput")
    d_outw = nc.dram_tensor("outw", [D, VSL], BF16, kind="ExternalInput")
    d_cosT = nc.dram_tensor("cosT", [128, S], BF16, kind="ExternalInput")
    d_sinT = nc.dram_tensor("sinT", [128, S], BF16, kind="ExternalInput")
    d_cosS = nc.dram_tensor("cosS", [128, 8, 128], BF16, kind="ExternalInput")
    d_sinS = nc.dram_tensor("sinS", [128, 8, 128], BF16, kind="ExternalInput")
    d_mask = nc.dram_tensor("maskT", [128, 4, 512], BF16, kind="ExternalInput")
    d_out = nc.dram_tensor("out", [S, VSL], F32, kind="ExternalOutput")

    with tile.TileContext(nc) as tc:
        with (
            tc.tile_pool(name="wp", bufs=1) as wp,
            tc.tile_pool(name="ap", bufs=1) as ap,
            tc.tile_pool(name="ap2", bufs=2) as ap2,
            tc.tile_pool(name="rt", bufs=4) as rt,
            tc.tile_pool(name="ps", bufs=8, space="PSUM") as ps,
            tc.tile_pool(name="dp", bufs=2, space="DRAM") as dp,
        ):
            _emit(nc, tc, wp, ap, ap2, rt, ps, dp, d_tok, d_emb, d_w, d_outw,
                  d_cosT, d_sinT, d_cosS, d_sinS, d_mask, d_out)

    nc.compile()
    return nc


def _emit(nc, tc, wp, ap, ap2, rt, ps, dp, d_tok, d_emb, d_w, d_outw,
          d_cosT, d_sinT, d_cosS, d_sinS, d_mask, d_out):
    TT = nc.vector.tensor_tensor
    ACT = nc.scalar.activation
    MM = nc.tensor.matmul
    AF = mybir.ActivationFunctionType
    OP = mybir.AluOpType

    # ---- persistent constants / weights in SBUF ----
    def load(name, shape, dtype, src):
        t = wp.tile(shape, dtype, tag=name)
        nc.sync.dma_start(out=t[:], in_=src)
        return t

    tok = load("tok", [128, 16], I32, d_tok[:, :])
    cosT = load("cosT", [128, S], BF16, d_cosT[:, :])
    sinT = load("sinT", [128, S], BF16, d_sinT[:, :])
    cosS = load("cosS", [128, 8, 128], BF16, d_cosS[:, :, :])
    sinS = load("sinS", [128, 8, 128], BF16, d_sinS[:, :, :])
    maskT = load("maskT", [128, 4, 512], BF16, d_mask[:, :, :])

    ident = wp.tile([128, 128], F32, tag="ident")
    make_identity(nc, ident[:])
    # constants used as activation bias APs
    for cval in (0.0, 1e-6):
        ct = wp.tile([128, 1], F32, tag=f"const{cval}")
        nc.vector.memset(ct[:], cval)
        nc.const_aps.aps[(F32, cval)] = ct[:]
    ones_cf = wp.tile([128, 1], F32, tag="ones_cf")
    nc.vector.memset(ones_cf[:], 1.0)
    ones_cb = wp.tile([128, 1], BF16, tag="ones_cb")
    nc.vector.memset(ones_cb[:], 1.0)
    ones_r = wp.tile([1, 128], F32, tag="ones_r")
    nc.vector.memset(ones_r[:], 1.0)

    def load_dd(name):
        """[D, D] weight -> SBUF [128, 2*D], chunk dch at [:, dch*D:(dch+1)*D]."""
        t = wp.tile([128, 2 * D], BF16, tag=name)
        for dch in range(2):
            nc.sync.dma_start(out=t[:, dch * D:(dch + 1) * D],
                              in_=d_w[name][dch * 128:(dch + 1) * 128, :])
        return t

    w = {}
    for l in range(L):
        for j in range(2):
            for nm in ("wq", "wk", "wv"):
                w[f"{nm}{l}h{j}"] = load_dd(f"{nm}{l}h{j}")
        t = wp.tile([128, 4 * D], BF16, tag=f"wo{l}")
        for dch in range(4):
            nc.sync.dma_start(out=t[:, dch * D:(dch + 1) * D],
                              in_=d_w[f"wo{l}"][dch * 128:(dch + 1) * 128, :])
        w[f"wo{l}"] = t
        for nm in ("ffw", "wg", "wl"):
            w[f"{nm}{l}"] = load_dd(f"{nm}{l}")
    for nm in ("fw", "fwg", "fwl"):
        w[nm] = load_dd(nm)

    # ---- embedding: gather from local table shard, 8-way ReduceScatter ----
    ers_in = dp.tile([2048, D], F32, tag="ers_in")
    for t16 in range(16):
        g = ap2.tile([128, D], F32, tag="x0g")
        nc.gpsimd.memset(g[:], 0.0)
        nc.gpsimd.indirect_dma_start(
            out=g[:], out_offset=None, in_=d_emb[:, :],
            in_offset=bass.IndirectOffsetOnAxis(ap=tok[:, t16:t16 + 1], axis=0),
            bounds_check=VSH - 1, oob_is_err=False)
        nc.sync.dma_start(out=ers_in[t16 * 128:(t16 + 1) * 128, :], in_=g[:])
    ers_out = dp.tile([SL, D], F32, tag="ers_out")
    nc.gpsimd.collective_compute(
        "ReduceScatter", OP.add, replica_groups=ALLG,
        ins=[ers_in[:].opt()], outs=[ers_out[:].opt()])

    xseq = ap.tile([128, 2 * D], F32, tag="xseq")  # [s-tile si, d]
    for si in range(2):
        nc.sync.dma_start(out=xseq[:, si * D:(si + 1) * D],
                          in_=ers_out[si * 128:(si + 1) * 128, :])
    # transpose to feature-major xT [128, dch*SL + s']
    xT = ap2.tile([128, 2 * SL], F32, tag="xT")
    for si in range(2):
        for dj in range(2):
            pt = ps.tile([128, 128], F32, tag="ps")
            nc.tensor.transpose(pt[:], xseq[:, si * D + dj * 128:
                                            si * D + dj * 128 + 128], ident[:])
            nc.scalar.activation(xT[:, dj * SL + si * 128:dj * SL + si * 128 + 128],
                                 pt[:], AF.Copy)

    # ---- helpers ----
    def rmsnorm(x_in, tag):
        """feature-major rmsnorm on the local slice. returns (xn_f32, xn_bf16)."""
        sq = ap.tile([128, 2 * SL], F32, tag="sq")
        TT(sq[:], x_in[:], x_in[:], op=OP.mult)
        msp = ps.tile([1, SL], F32, tag="ps")
        for dch in range(2):
            MM(msp[:], lhsT=ones_cf[:], rhs=sq[:, dch * SL:(dch + 1) * SL],
               start=(dch == 0), stop=(dch == 1))
        lnm = ap.tile([1, SL], F32, tag="lnm")
        ACT(lnm[:], msp[:], AF.Ln, bias=1e-6, scale=1.0 / D)
        rstd = ap.tile([1, SL], F32, tag="rstd")
        ACT(rstd[:], lnm[:], AF.Exp, scale=-0.5)
        bc = ps.tile([128, SL], F32, tag="ps")
        MM(bc[:], lhsT=ones_r[:], rhs=rstd[:], start=True, stop=True)
        xn = ap.tile([128, 2 * SL], F32, tag=tag)
        xnb = ap.tile([128, 2 * SL], BF16, tag=tag + "b")
        for dch in range(2):
            TT(xn[:, dch * SL:(dch + 1) * SL], x_in[:, dch * SL:(dch + 1) * SL],
               bc[:], op=OP.mult)
            nc.vector.tensor_copy(xnb[:, dch * SL:(dch + 1) * SL],
                                  xn[:, dch * SL:(dch + 1) * SL])
        return xn, xnb

    def allgather_feat(src_bf16, tag):
        """AllGather the [128, 2*SL] bf16 feature-major slice within the batch
        group -> [128, 2*S] feature-major full-batch tile."""
        agi = dp.tile([2 * 128, SL], BF16, tag=tag + "_agi")
        for dch in range(2):
            nc.sync.dma_start(out=agi[dch * 128:(dch + 1) * 128, :],
                              in_=src_bf16[:, dch * SL:(dch + 1) * SL])
        ago = dp.tile([4, 2 * 128, SL], BF16, tag=tag + "_ago")
        nc.gpsimd.collective_compute(
            "AllGather", OP.bypass, replica_groups=GROUPS,
            ins=[agi[:].opt()], outs=[ago[:].opt()])
        full = ap2.tile([128, 2 * S], BF16, tag=tag)
        for dch in range(2):
            for g4 in range(4):
                nc.sync.dma_start(
                    out=full[:, dch * S + g4 * SL:dch * S + (g4 + 1) * SL],
                    in_=ago[g4, dch * 128:(dch + 1) * 128, :])
        return full

    def rope_feat(raw, out, tag):
        """RoPE on a feature-major [128, 2*S] bf16 tensor (A block then B)."""
        A, Bq = raw[:, 0:S], raw[:, S:2 * S]
        t1 = rt.tile([128, S], BF16, tag="rtmp")
        t2 = rt.tile([128, S], BF16, tag="rtmp")
        TT(t1[:], A, cosT[:], op=OP.mult)
        TT(t2[:], Bq, sinT[:], op=OP.mult)
        TT(out[:, 0:S], t1[:], t2[:], op=OP.add)
        t3 = rt.tile([128, S], BF16, tag="rtmp")
        t4 = rt.tile([128, S], BF16, tag="rtmp")
        TT(t3[:], Bq, cosT[:], op=OP.mult)
        TT(t4[:], A, sinT[:], op=OP.mult)
        TT(out[:, S:2 * S], t3[:], t4[:], op=OP.subtract)

    def ffn(x_bf, wf, wg_, wl_, out_dtype, out_tag):
        """x_bf [128, 2*SL] bf16 -> swiglu(x@wf) [128, 2*SL] feature-major."""
        hps = []
        for ech in range(2):
            hp = ps.tile([128, SL], F32, tag="ps")
            for dch in range(2):
                MM(hp[:], lhsT=wf[:, dch * D + ech * 128:dch * D + ech * 128 + 128],
                   rhs=x_bf[:, dch * SL:(dch + 1) * SL],
                   start=(dch == 0), stop=(dch == 1))
            hps.append(hp)
        hb = ap.tile([128, 2 * SL], BF16, tag="hb")
        for ech in range(2):
            ACT(hb[:, ech * SL:(ech + 1) * SL], hps[ech][:], AF.Copy)
        gps, lps = [], []
        for wmat, acc in ((wg_, gps), (wl_, lps)):
            for ech in range(2):
                p = ps.tile([128, SL], F32, tag="ps")
                for dch in range(2):
                    MM(p[:], lhsT=wmat[:, dch * D + ech * 128:dch * D + ech * 128 + 128],
                       rhs=hb[:, dch * SL:(dch + 1) * SL],
                       start=(dch == 0), stop=(dch == 1))
                acc.append(p)
        fo = ap.tile([128, 2 * SL], out_dtype, tag=out_tag)
        eN = ap.tile([128, 2 * SL], F32, tag="eN")
        den = ap.tile([128, 2 * SL], F32, tag="den")
        rc2 = ap.tile([128, 2 * SL], F32, tag="rc2")
        swt = ap.tile([128, 2 * SL], F32, tag="swt")
        for ech in range(2):
            sl = slice(ech * SL, (ech + 1) * SL)
            ACT(eN[:, sl], gps[ech][:], AF.Exp, scale=-1.0)
            nc.vector.tensor_scalar_add(den[:, sl], eN[:, sl], 1.0)
            nc.vector.reciprocal(rc2[:, sl], den[:, sl])
            nc.vector.scalar_tensor_tensor(
                out=swt[:, sl], in0=gps[ech][:], scalar=1.0, in1=rc2[:, sl],
                op0=OP.mult, op1=OP.mult)
            TT(fo[:, sl], swt[:, sl], lps[ech][:], op=OP.mult)
        return fo

    # ---- transformer layers ----
    x_cur = xT
    for l in range(L):
        xn, xnb = rmsnorm(x_cur, f"xn")
        xnF = allgather_feat(xnb, "xnF")

        attnS = ap.tile([128, 2 * S], F32, tag="attnS")
        for j in range(2):  # heads
            # --- q, k projections (feature-major, weight-stationary) ---
            qraw = ap.tile([128, 2 * S], BF16, tag="qraw")
            kraw = ap.tile([128, 2 * S], BF16, tag="kraw")
            for wmat, dst in ((w[f"wq{l}h{j}"], qraw), (w[f"wk{l}h{j}"], kraw)):
                for ech in range(2):
                    for sch in range(2):
                        p = ps.tile([128, 512], F32, tag="ps")
                        for dch in range(2):
                            MM(p[:],
                               lhsT=wmat[:, dch * D + ech * 128:dch * D + ech * 128 + 128],
                               rhs=xnF[:, dch * S + sch * 512:dch * S + (sch + 1) * 512],
                               start=(dch == 0), stop=(dch == 1))
                        ACT(dst[:, ech * S + sch * 512:ech * S + (sch + 1) * 512],
                            p[:], AF.Copy)
            qT = ap.tile([128, 2 * S], BF16, tag="qT")
            kT = ap.tile([128, 2 * S], BF16, tag="kT")
            rope_feat(qraw, qT, "q")
            rope_feat(kraw, kT, "k")

            # --- v projection (sequence-major, xn-stationary) ---
            vraw = ap.tile([128, 8, D], BF16, tag="vraw")
            for st in range(8):
                p = ps.tile([128, D], F32, tag="ps")
                for dch in range(2):
                    MM(p[:],
                       lhsT=xnF[:, dch * S + st * 128:dch * S + (st + 1) * 128],
                       rhs=w[f"wv{l}h{j}"][:, dch * D:(dch + 1) * D],
                       start=(dch == 0), stop=(dch == 1))
                nc.vector.tensor_copy(vraw[:, st, :], p[:])
            # RoPE on v (gpsimd; pairs are columns 0:128 / 128:256)
            vR = ap.tile([128, 8, D], BF16, tag="vR")
            vA, vB = vraw[:, :, 0:128], vraw[:, :, 128:256]
            g1 = rt.tile([128, 8, 128], BF16, tag="vtmp")
            g2 = rt.tile([128, 8, 128], BF16, tag="vtmp")
            nc.gpsimd.tensor_tensor(g1[:], vA, cosS[:], op=OP.mult)
            nc.gpsimd.tensor_tensor(g2[:], vB, sinS[:], op=OP.mult)
            nc.gpsimd.tensor_tensor(vR[:, :, 0:128], g1[:], g2[:], op=OP.add)
            g3 = rt.tile([128, 8, 128], BF16, tag="vtmp")
            g4 = rt.tile([128, 8, 128], BF16, tag="vtmp")
            nc.gpsimd.tensor_tensor(g3[:], vB, cosS[:], op=OP.mult)
            nc.gpsimd.tensor_tensor(g4[:], vA, sinS[:], op=OP.mult)
            nc.gpsimd.tensor_tensor(vR[:, :, 128:256], g3[:], g4[:],
                                    op=OP.subtract)

            # --- scores^T blocks + exp + causal mask ---
            E = ap.tile([128, E_WIDTH], BF16, tag="E")
            for jb in range(8):
                scp = {ic: ps.tile([128, 512], F32, tag="ps",
                                   name=f"scp{l}_{j}_{jb}_{ic}")
                       for ic in _valid_ic(jb)}
                for dch in range(2):
                    for ic in _valid_ic(jb):
                        MM(scp[ic][:],
                           lhsT=qT[:, dch * S + jb * 128:dch * S + (jb + 1) * 128],
                           rhs=kT[:, dch * S + ic * 512:dch * S + (ic + 1) * 512],
                           start=(dch == 0), stop=(dch == 1))
                for ic in _valid_ic(jb):
                    col = _e_col(jb, ic)
                    ACT(E[:, col:col + 512], scp[ic][:], AF.Exp, scale=NEG_SCALE)
                # diagonal block mask (block ic == jb//4)
                cold = _e_col(jb, jb // 4)
                TT(E[:, cold:cold + 512], E[:, cold:cold + 512],
                   maskT[:, jb % 4, :], op=OP.mult)

            # --- row sums over j (ones-matmul), broadcast, reciprocal ---
            rsS = ap.tile([1, S], F32, tag="rsS")
            rcp = ap.tile([128, S], F32, tag="rcp")
            for ic in range(2):
                rp = ps.tile([1, 512], F32, tag="ps")
                jbs = [jb for jb in range(8) if ic in _valid_ic(jb)]
                for n, jb in enumerate(jbs):
                    col = _e_col(jb, ic)
                    MM(rp[:], lhsT=ones_cb[:], rhs=E[:, col:col + 512],
                       start=(n == 0), stop=(n == len(jbs) - 1))
                ACT(rsS[:, ic * 512:(ic + 1) * 512], rp[:], AF.Copy)
                bp = ps.tile([128, 512], F32, tag="ps")
                MM(bp[:], lhsT=ones_r[:], rhs=rsS[:, ic * 512:(ic + 1) * 512],
                   start=True, stop=True)
                nc.vector.reciprocal(rcp[:, ic * 512:(ic + 1) * 512], bp[:])

            # --- attention * V (o feature-major), normalize ---
            avp = {(dpr, ic): ps.tile([128, 512], F32, tag="ps",
                                      name=f"avp{l}_{j}_{dpr}_{ic}")
                   for dpr in range(2) for ic in range(2)}
            for jb in range(8):
                for dpr in range(2):
                    for ic in _valid_ic(jb):
                        col = _e_col(jb, ic)
                        MM(avp[(dpr, ic)][:],
                           lhsT=vR[:, jb, dpr * 128:(dpr + 1) * 128],
                           rhs=E[:, col:col + 512],
                           start=(jb == 0), stop=(jb == (3 if ic == 0 else 7)))
            oN = ap.tile([128, 2 * S], BF16, tag="oN")
            for dpr in range(2):
                for ic in range(2):
                    sl = slice(dpr * S + ic * 512, dpr * S + (ic + 1) * 512)
                    TT(oN[:, sl], avp[(dpr, ic)][:],
                       rcp[:, ic * 512:(ic + 1) * 512], op=OP.mult)

            # --- Wo partial (accumulate heads in SBUF) ---
            for ech in range(2):
                for sch in range(2):
                    p = ps.tile([128, 512], F32, tag="ps")
                    for dpr in range(2):
                        MM(p[:],
                           lhsT=w[f"wo{l}"][:, (2 * j + dpr) * D + ech * 128:
                                            (2 * j + dpr) * D + ech * 128 + 128],
                           rhs=oN[:, dpr * S + sch * 512:dpr * S + (sch + 1) * 512],
                           start=(dpr == 0), stop=(dpr == 1))
                    sl = slice(ech * S + sch * 512, ech * S + (sch + 1) * 512)
                    if j == 0:
                        ACT(attnS[:, sl], p[:], AF.Copy)
                    else:
                        TT(attnS[:, sl], attnS[:, sl], p[:], op=OP.add)

        # --- ReduceScatter attention output, residual, rms2, FFN ---
        rsi = dp.tile([4, 2 * 128, SL], F32, tag="arsi")
        for g4 in range(4):
            for ech in range(2):
                nc.sync.dma_start(
                    out=rsi[g4, ech * 128:(ech + 1) * 128, :],
                    in_=attnS[:, ech * S + g4 * SL:ech * S + (g4 + 1) * SL])
        rso = dp.tile([2 * 128, SL], F32, tag="arso")
        nc.gpsimd.collective_compute(
            "ReduceScatter", OP.add, replica_groups=GROUPS,
            ins=[rsi[:].opt()], outs=[rso[:].opt()])
        aS = ap.tile([128, 2 * SL], F32, tag="aS")
        for ech in range(2):
            nc.sync.dma_start(out=aS[:, ech * SL:(ech + 1) * SL],
                              in_=rso[ech * 128:(ech + 1) * 128, :])
        x2 = ap2.tile([128, 2 * SL], F32, tag="xT")
        TT(x2[:], xn[:], aS[:], op=OP.add)

        xn2, xn2b = rmsnorm(x2, "xn2")
        fo = ffn(xn2b, w[f"ffw{l}"], w[f"wg{l}"], w[f"wl{l}"], F32, "fo")
        x3 = ap2.tile([128, 2 * SL], F32, tag="xT")
        TT(x3[:], xn2[:], fo[:], op=OP.add)
        x_cur = x3

    # ---- final head ----
    xfb = ap.tile([128, 2 * SL], BF16, tag="xfb")
    nc.vector.tensor_copy(xfb[:], x_cur[:])
    fT = ffn(xfb, w["fw"], w["fwg"], w["fwl"], BF16, "fT")
    fF = allgather_feat(fT, "fF")

    VHW = VSL // 2   # 4000 vocab cols per half
    VC = 500         # free-dim chunk (8 per half)
    owc = ap.tile([128, 2 * VHW], BF16, tag="owc")
    for vh in range(2):
        for dch in range(2):
            nc.sync.dma_start(
                out=owc[:, dch * VHW:(dch + 1) * VHW],
                in_=d_outw[dch * 128:(dch + 1) * 128,
                           vh * VHW:(vh + 1) * VHW])
        for st in range(8):
            pls = [ps.tile([128, VC], F32, tag="ps", name=f"pls{vh}_{st}_{i}")
                   for i in range(8)]
            for dch in range(2):
                for vc in range(8):
                    MM(pls[vc][:],
                       lhsT=fF[:, dch * S + st * 128:dch * S + (st + 1) * 128],
                       rhs=owc[:, dch * VHW + vc * VC:dch * VHW + (vc + 1) * VC],
                       start=(dch == 0), stop=(dch == 1))
            for vc in range(8):
                oc = ap2.tile([128, VC], F32, tag="oc", bufs=4)
                if vc % 2 == 0:
                    ACT(oc[:], pls[vc][:], AF.Copy)
                else:
                    nc.vector.tensor_copy(oc[:], pls[vc][:])
                nc.sync.dma_start(
                    out=d_out[st * 128:(st + 1) * 128,
                              vh * VHW + vc * VC:vh * VHW + (vc + 1) * VC],
                    in_=oc[:])


# ----------------------------------------------------------------------------
# Host side
# ----------------------------------------------------------------------------

def _rope_tables_np():
    i = np.arange(D // 2, dtype=np.float32)
    theta = (10000.0 ** (-2.0 * (i - 1.0) / D)).astype(np.float32)
    ang = np.arange(S, dtype=np.float32)[:, None] * theta[None, :]
    return np.cos(ang).astype(np.float32), np.sin(ang).astype(np.float32)


def _prep_inputs(tokens, params):
    """Build the 8 per-core input maps."""
    perm = np.concatenate([np.arange(0, D, 2), np.arange(1, D, 2)])
    cos, sin = _rope_tables_np()  # [S, 128]

    def b16(x):
        return np.asarray(x, dtype=np.float32).astype(_bf)

    cosT = b16(cos.T)                      # [128, S]
    sinT = b16(sin.T)
    cosSe = b16(cos.reshape(8, 128, 128).transpose(1, 0, 2))  # [p, st, i]
    sinSe = b16(sin.reshape(8, 128, 128).transpose(1, 0, 2))
    # mask[p, o, f] = 1.0 if f >= p + o*128 else 0
    f = np.arange(512)[None, None, :]
    p = np.arange(128)[:, None, None]
    o = np.arange(4)[None, :, None]
    maskT = (f >= p + o * 128).astype(np.float32).astype(_bf)

    def z(x):
        assert np.all(np.asarray(x) == 0.0), "nonzero bias unsupported"

    def one(x):
        assert np.all(np.asarray(x) == 1.0), "non-unit rms scale unsupported"

    lp = params["layers"]
    for pl in lp:
        for k in ("bq", "bk", "bv", "bo", "ff_b", "sg_bg", "sg_bl"):
            z(pl[k])
        one(pl["rms1"])
        one(pl["rms2"])
    for k in ("f_b", "f_sg_bg", "f_sg_bl", "out_b"):
        z(params[k])

    emb = np.asarray(params["embed"], dtype=np.float32)
    tokens = np.asarray(tokens, dtype=np.int32).reshape(-1)  # [2048]

    in_maps = []
    for c in range(NCORES):
        p4 = c % 4
        m = {}
        loc = tokens - c * VSH
        loc = np.where((loc >= 0) & (loc < VSH), loc, VSH).astype(np.int32)
        m["tok"] = loc.reshape(16, 128).T.copy()  # [p, t16]
        m["emb"] = emb[c * VSH:(c + 1) * VSH]
        for l in range(L):
            pl = lp[l]
            for j in range(2):
                h = 2 * p4 + j
                m[f"wq{l}h{j}"] = b16(np.asarray(pl["Wq"])[h][:, perm])
                m[f"wk{l}h{j}"] = b16(np.asarray(pl["Wk"])[h][:, perm])
                m[f"wv{l}h{j}"] = b16(np.asarray(pl["Wv"])[h][:, perm])
            wo = np.asarray(pl["Wo"])  # [H*D, D]
            blocks = []
            for j in range(2):
                h = 2 * p4 + j
                blocks.append(wo[h * D:(h + 1) * D][perm, :])
            m[f"wo{l}"] = b16(np.concatenate(blocks, axis=0))
            m[f"ffw{l}"] = b16(pl["ff_W"])
            m[f"wg{l}"] = b16(pl["sg_Wg"])
            m[f"wl{l}"] = b16(pl["sg_Wl"])
        m["fw"] = b16(params["f_W"])
        m["fwg"] = b16(params["f_sg_Wg"])
        m["fwl"] = b16(params["f_sg_Wl"])
        m["outw"] = b16(np.asarray(params["out_W"])[:, p4 * VSL:(p4 + 1) * VSL])
        m["cosT"] = cosT
        m["sinT"] = sinT
        m["cosS"] = cosSe
        m["sinS"] = sinSe
        m["maskT"] = maskT
        in_maps.append(m)
    return in_maps


def _get_runner():
    """Build (once) a cached jitted SPMD executable for the compiled module.

    Mirrors bass2jax.run_bass_via_pjrt's multi-core path, but keeps the jitted
    function and the (device-resident, undonated) zero output buffers alive so
    repeat calls skip retracing and the 262MB zero upload.  Safe because the
    kernel writes every element of its output tensor.
    """
    if "runner" in _CACHED:
        return _CACHED["runner"]
    import jax
    from jax.sharding import Mesh, PartitionSpec
    from jax.experimental.shard_map import shard_map
    from concourse import bass2jax, mybir as _mb

    if "nc" not in _CACHED:
        _CACHED["nc"] = build_nc()
    nc = _CACHED["nc"]
    bass2jax.install_neuronx_cc_hook()
    assert nc.dbg_addr is None
    part_name = (nc.partition_id_tensor.name
                 if nc.partition_id_tensor is not None else None)

    in_names, out_names, out_avals, zero_outs = [], [], [], []
    for alloc in nc.m.functions[0].allocations:
        if not isinstance(alloc, _mb.MemoryLocationSet):
            continue
        name = alloc.memorylocations[0].name
        if alloc.kind == "ExternalInput":
            if name != part_name:
                in_names.append(name)
        elif alloc.kind == "ExternalOutput":
            out_names.append(name)
            shape = tuple(alloc.tensor_shape)
            dtype = _mb.dt.np(alloc.dtype)
            out_avals.append(jax.core.ShapedArray(shape, dtype))
            zero_outs.append(np.zeros((NCORES * shape[0], *shape[1:]), dtype))
    n_params = len(in_names)
    all_names = in_names + out_names
    if part_name is not None:
        all_names = all_names + [part_name]

    def _body(*args):
        operands = list(args)
        if part_name is not None:
            operands.append(bass2jax.partition_id_tensor())
        outs = bass2jax._bass_exec_p.bind(
            *operands, out_avals=tuple(out_avals), in_names=tuple(all_names),
            out_names=tuple(out_names), lowering_input_output_aliases=(),
            sim_require_finite=True, sim_require_nnan=True, nc=nc)
        return tuple(outs)

    devices = jax.devices()[:NCORES]
    mesh = Mesh(np.asarray(devices), ("core",))
    nargs = n_params + len(out_names)
    sharded = jax.jit(
        shard_map(_body, mesh=mesh, in_specs=(PartitionSpec("core"),) * nargs,
                  out_specs=(PartitionSpec("core"),) * len(out_names),
                  check_rep=False),
        keep_unused=True)
    sharding = jax.sharding.NamedSharding(mesh, PartitionSpec("core"))
    dev_zeros = [jax.device_put(z, sharding) for z in zero_outs]

    def run(in_maps, dev_inputs=None):
        if dev_inputs is None:
            dev_inputs = put_inputs(in_maps)
        out_arrs = sharded(*dev_inputs, *dev_zeros)
        return out_arrs

    def put_inputs(in_maps):
        concat = [np.concatenate([np.asarray(in_maps[c][nm])
                                  for c in range(NCORES)], axis=0)
                  for nm in in_names]
        return [jax.device_put(a, sharding) for a in concat]

    _CACHED["runner"] = (run, put_inputs, out_names, out_avals)
    return _CACHED["runner"]


def kernel(tokens, params):
    run, put_inputs, out_names, out_avals = _get_runner()
    in_maps = _prep_inputs(tokens, params)
    out_arrs = run(in_maps)
    oi = out_names.index("out")
    full = np.asarray(out_arrs[oi]).reshape(NCORES, S, VSL)
    out = np.empty((B, S, V), dtype=np.float32)
    for c in range(NCORES):
        b, p4 = c // 4, c % 4
        out[b, :, p4 * VSL:(p4 + 1) * VSL] = full[c]
    return out
